# revision 41
# baseline (speedup 1.0000x reference)
"""SPP (spatial pyramid pooling) kernel for Trainium2, 8 NeuronCores.

Input  x  : [16, 256, 64, 64] f32
Output    : [16, 5376, 13, 13] f32

Math: windows are 16x16 at stride 4 -> 13x13 window grid. Levels use
sub-cells of 16/8/4 pixels, all aligned to multiples of 4, so everything
reduces to the non-overlapping 4x4 block-max P2 [16,16] per (b,c) image:
  lvl2 plane (q,r) = P2[q+i, r+j]              (16 planes of 13x13)
  P1 = 2x2 stride-1 max of P2 -> [15,15];  lvl1 plane (q,r) = P1[2q+i, 2r+j]
  P0 = 4x4 stride-1 max of P2 -> [13,13];  lvl0 plane    = P0
Output channel order: [lvl0: c][lvl1: c*4+q*2+r][lvl2: c*16+q*4+r].

Precision: the whole pipeline runs in bf16 (host converts f32->bf16 in,
bf16->f32 out).  Max-pooling is exact in any dtype, so the only error is
input rounding (<= 2^-9 relative), far inside the 2e-2 gate, and it
halves HBM traffic: 15.7 MB -> 7.8 MB per core (floor ~22 us).

Sharding: data-parallel over batch; each of 8 cores handles 2 samples as
4 tiles of 128 (b,c)-images on partitions.  Engine split per tile:
VectorE does the pairwise max trees plus the lvl1 gathers (all
tensor_tensor ops -- 1-port mode, never contends with GpSimd; a
same-operand tensor_max is a gather-copy at 2 elem/cyc), ScalarE stages
the 16 lvl2 planes (4 gather-copies), GpSimd issues the small SWDGE
stores.  Loads on the SP HWDGE ring (first tile as two halves, one per
HWDGE ring, so their transfers and sems overlap), one merged lvl2 store per
tile on the ACT ring (more stores -> HWDGE sem-lane stalls, measured
+3.5 us).  On the last tile the staging fans out (q0/q1 ACT, q2/q3 DVE)
and the three final store dispatches go on three different rings.

Measured: 51.5 us (f32 baseline) -> 37.8-38.3 us.  Breakdown: ~6.8 us
fixed NEFF preamble, ~8 us load-latency fill, ~21 us byte-bound DMA
stream (358 GB/s/core HBM), ~3.2 us final receipt + exit barrier.
"""

import sys

for _p in ("/opt/trn_rl_repo", "/opt/trn_rl_repo/concourse"):
    if _p not in sys.path:
        sys.path.insert(0, _p)

import numpy as np
import ml_dtypes

N_CORES = 8
BS, C, H, W = 16, 256, 64, 64
B_PER_CORE = BS // N_CORES  # 2
OH = OW = 13
CBLK = 2  # channel blocks of 128 per sample

_nc_cache = {}


def _build_nc(finalize=True):
    import concourse.bacc as bacc
    import concourse.mybir as mybir
    from concourse import tile
    from concourse.ap import AP as APc

    bf16 = mybir.dt.bfloat16
    # Bacc (not bare Bass): its finalize() runs generate_event_semaphores,
    # which splits multi-sem sync waits that walrus cannot encode.
    nc = bacc.Bacc("TRN2", target_bir_lowering=False)
    x = nc.dram_tensor("x", [B_PER_CORE, C, H, W], bf16, kind="ExternalInput")
    o = nc.dram_tensor("out", [B_PER_CORE, 21 * C, OH, OW], bf16, kind="ExternalOutput")

    def overlap(tap, start, dims):
        """Strided (possibly overlapping) free-dim view of a tile AP,
        starting at free-offset `start`.  Max 3 free dims (ISA limit)."""
        base = tap[:, start:]
        part = list(base.ap[0])
        return APc(
            tensor=base.tensor,
            offset=base.offset,
            ap=[part] + [[s, n] for (s, n) in dims],
        )

    with tile.TileContext(nc) as tc:
        with tc.tile_pool(name="sbuf", bufs=2) as pool:
            first = True
            for b in range(B_PER_CORE):
                for cb in range(CBLK):
                    cs = slice(cb * 128, (cb + 1) * 128)
                    r4 = pool.tile([128, 1024], bf16, tag="r4")
                    if first:
                        # Pipeline fill: two half loads, one per HWDGE ring
                        # (SP + ACT), so their transfers and completion sems
                        # overlap.  Two halves beat four quarters (fewer
                        # DMAs/ops/sems: 35.5 vs 37.8 us) and beat a single
                        # whole-tile load (36.3 us) -- the last chunk's sem
                        # latency dominates the fill either way, and fewer
                        # dispatches pull the later tile loads earlier.
                        first = False
                        for qt, (r0, rows) in enumerate(((0, 32), (32, 32))):
                            xq = pool.tile(
                                [128, rows * W], bf16, tag=f"xq{qt}", bufs=1
                            )
                            ring = nc.sync if qt % 2 == 0 else nc.scalar
                            ring.dma_start(
                                out=xq[:],
                                in_=x[b, cs, r0 : r0 + rows].rearrange(
                                    "c h w -> c (h w)"
                                ),
                            )
                            bq = pool.tile(
                                [128, rows * W // 2], bf16, tag=f"bq{qt}", bufs=1
                            )
                            xqv = xq.rearrange("p (a t c) -> p a t c", t=2, c=W)
                            nc.vector.tensor_max(
                                out=bq.rearrange("p (a c) -> p a c", c=W),
                                in0=xqv[:, :, 0, :],
                                in1=xqv[:, :, 1, :],
                            )
                            bqv = bq.rearrange("p (a t c) -> p a t c", t=2, c=W)
                            nc.vector.tensor_max(
                                out=r4[:, r0 * 16 : (r0 + rows) * 16].rearrange(
                                    "p (a c) -> p a c", c=W
                                ),
                                in0=bqv[:, :, 0, :],
                                in1=bqv[:, :, 1, :],
                            )
                    else:
                        # bufs=3 (no slot reuse among tiles 1-3): keeps loads
                        # early and waits trivial.  (Splitting any of these
                        # loads regresses 1-5 us: an extra DMA shifts the
                        # round-robin HWDGE sem-lane assignment for every
                        # later DMA and introduces event-sem stalls.)
                        xt = pool.tile([128, H * W], bf16, tag="xt", bufs=3)
                        nc.sync.dma_start(
                            out=xt[:],
                            in_=x[b, cs].rearrange("c h w -> c (h w)"),
                        )
                        b1 = pool.tile([128, 2048], bf16, tag="b1")
                        xv = xt.rearrange("p (a t c) -> p a t c", t=2, c=W)
                        nc.vector.tensor_max(
                            out=b1.rearrange("p (a c) -> p a c", c=W),
                            in0=xv[:, :, 0, :],
                            in1=xv[:, :, 1, :],
                        )
                        bv = b1.rearrange("p (a t c) -> p a t c", t=2, c=W)
                        nc.vector.tensor_max(
                            out=r4.rearrange("p (a c) -> p a c", c=W),
                            in0=bv[:, :, 0, :],
                            in1=bv[:, :, 1, :],
                        )
                    # 4-col max: [16,64] -> P2 [16,16].  First stage pairs
                    # (k, k+2) via run-of-2 step-1 views (packable 2x mode);
                    # d[g,u] = max(r4[4g+u], r4[4g+u+2]); p2[g] = max(d[g,0],
                    # d[g,1]).
                    c1 = pool.tile([128, 512], bf16, tag="c1")
                    nc.vector.tensor_max(
                        out=c1.rearrange("p (g u) -> p g u", u=2),
                        in0=overlap(r4, 0, [(4, 256), (1, 2)]),
                        in1=overlap(r4, 2, [(4, 256), (1, 2)]),
                    )
                    p2 = pool.tile([128, 256], bf16, tag="p2")
                    nc.vector.tensor_max(out=p2[:], in0=c1[:, 0::2], in1=c1[:, 1::2])

                    # bufs=3: with 2, tile t+2's compute waits on tile t's
                    # stores releasing the stage slot, which starves the
                    # store stream mid-kernel (measured 65% SDMA dip).
                    # bufs=4 measured neutral (38.1 us, within noise).
                    stage = pool.tile([128, 21 * OH * OW], bf16, tag="stage", bufs=3)

                    lvl2_dst = o[b, 1280 + cb * 2048 : 1280 + (cb + 1) * 2048].rearrange(
                        "(c f) h w -> c (f h w)", f=16
                    )
                    last = b == B_PER_CORE - 1 and cb == CBLK - 1

                    def lvl2_src(q):
                        return overlap(p2, q * 16, [(1, 4), (16, 13), (1, 13)])

                    def lvl2_dstap(q):
                        return stage[:, (5 + 4 * q) * 169 : (9 + 4 * q) * 169]

                    # lvl2: 16 shifted 13x13 windows of P2 -> stage[845:3549]
                    # (split over q: ISA mem patterns allow at most 3 free
                    # dims).  Mid tiles: all 4 q-chunks on ACT with ONE
                    # merged store -- the HWDGE dma_start dispatch overlaps
                    # the ACT pipe, and one dispatch keeps the sequencer
                    # stream short.  Last tile: q0/q1 on ACT (store as soon
                    # as both staged), q2/q3 as DVE same-operand tensor_max
                    # gathers AFTER the small-store producers so the SP-ring
                    # smalls dispatch early; their store goes on the idle SP
                    # ring, dispatched in parallel with ACT's.
                    if not last:
                        for q in range(4):
                            nc.scalar.copy(out=lvl2_dstap(q), in_=lvl2_src(q))
                        # Mid-tile stores go on the SYNC ring: its FIFO
                        # queues them BEHIND the remaining loads, which
                        # acts as a loads-first priority (no HW QoS knob
                        # exists) -- loads finish sooner, and the deferred
                        # store backlog then drains at full rate.
                        nc.sync.dma_start(
                            out=lvl2_dst[:],
                            in_=stage[:, 5 * 169 : 21 * 169],
                        )
                    else:
                        for q in range(2):
                            nc.scalar.copy(out=lvl2_dstap(q), in_=lvl2_src(q))
                        nc.scalar.dma_start(
                            out=lvl2_dst[:, : 8 * 169],
                            in_=stage[:, 5 * 169 : 13 * 169],
                        )
                    # P1 = 2x2 stride-1 max of P2 -> [15,15]
                    t1 = pool.tile([128, 240], bf16, tag="t1")
                    p2m = p2.rearrange("p (h w) -> p h w", w=16)
                    nc.vector.tensor_max(
                        out=t1.rearrange("p (h w) -> p h w", w=15),
                        in0=p2m[:, :, 0:15],
                        in1=p2m[:, :, 1:16],
                    )
                    p1 = pool.tile([128, 225], bf16, tag="p1")
                    nc.vector.tensor_max(
                        out=p1[:], in0=t1[:, 0:225], in1=t1[:, 15:240]
                    )
                    # lvl1: 4 shifted 13x13 windows of P1 (stride 2) ->
                    # stage[169:845].  Same-operand tensor_max on DVE =
                    # gather-copy in packed 2x mode (~0.33 us/op).  (Staging
                    # any tile's lvl1 on ACT instead measured +2 us: the
                    # extra ACT-stream ops delay that tile's store path.)
                    for q in range(2):
                        src = lambda: overlap(p1, q * 30, [(2, 2), (15, 13), (1, 13)])
                        nc.vector.tensor_max(
                            out=stage[:, (1 + 2 * q) * 169 : (3 + 2 * q) * 169],
                            in0=src(),
                            in1=src(),
                        )
                    # P0 = 4x4 stride-1 max of P2 = 2x2 stride-2 max of P1
                    t2 = pool.tile([128, 195], bf16, tag="t2")
                    p1m = p1.rearrange("p (h w) -> p h w", w=15)
                    nc.vector.tensor_max(
                        out=t2.rearrange("p (h w) -> p h w", w=13),
                        in0=p1m[:, :, 0:13],
                        in1=p1m[:, :, 2:15],
                    )
                    nc.vector.tensor_max(
                        out=stage[:, 0:169], in0=t2[:, 0:169], in1=t2[:, 26:195]
                    )
                    # Small stores: SWDGE (GpSimd) keeps the HWDGE DMA count
                    # low.  On the last tile, spread the three remaining
                    # store dispatches across three rings (Pool/SP/ACT) so
                    # they fire concurrently instead of serializing ~0.7 us
                    # apiece behind one sequencer.
                    nc.gpsimd.dma_start(
                        out=o[b, 256 + cb * 512 : 256 + (cb + 1) * 512].rearrange(
                            "(c f) h w -> c (f h w)", f=4
                        ),
                        in_=stage[:, 169 : 5 * 169],
                    )
                    small = nc.sync if last else nc.gpsimd
                    small.dma_start(
                        out=o[b, cs].rearrange("c h w -> c (h w)"),
                        in_=stage[:, 0:169],
                    )
                    if last:
                        # Tail lvl2 q2/q3: DVE gathers (the scheduler runs
                        # them interleaved with the t1..p0 chain), store on
                        # the ACT ring which is idle after dma(q0q1).
                        # (Merging the last tile's two stores into one
                        # measured +3.5 us: the single store waits on all
                        # four chunks incl. the late DVE gathers.)
                        for q in (2, 3):
                            nc.vector.tensor_max(
                                out=lvl2_dstap(q), in0=lvl2_src(q), in1=lvl2_src(q)
                            )
                        nc.scalar.dma_start(
                            out=lvl2_dst[:, 8 * 169 :],
                            in_=stage[:, 13 * 169 : 21 * 169],
                        )
    if finalize:
        nc.finalize()
    return nc


def get_nc():
    if "nc" not in _nc_cache:
        _nc_cache["nc"] = _build_nc()
    return _nc_cache["nc"]


def kernel(x: np.ndarray, _trace: bool = False):
    from concourse.bass_utils import run_bass_kernel_spmd

    x = np.asarray(x)
    assert x.shape == (BS, C, H, W), x.shape
    xb = np.ascontiguousarray(x).astype(ml_dtypes.bfloat16)
    nc = get_nc()
    in_maps = [
        {"x": xb[c * B_PER_CORE : (c + 1) * B_PER_CORE]} for c in range(N_CORES)
    ]
    res = run_bass_kernel_spmd(
        nc, in_maps, core_ids=list(range(N_CORES)), trace=_trace
    )
    out = np.concatenate(
        [r["out"].astype(np.float32) for r in res.results], axis=0
    )
    if _trace:
        return out, res
    return out


# revision 42
# speedup vs baseline: 1.0005x; 1.0005x over previous
"""SPP (spatial pyramid pooling) kernel for Trainium2, 8 NeuronCores.

Input  x  : [16, 256, 64, 64] f32
Output    : [16, 5376, 13, 13] f32

Math: windows are 16x16 at stride 4 -> 13x13 window grid. Levels use
sub-cells of 16/8/4 pixels, all aligned to multiples of 4, so everything
reduces to the non-overlapping 4x4 block-max P2 [16,16] per (b,c) image:
  lvl2 plane (q,r) = P2[q+i, r+j]              (16 planes of 13x13)
  P1 = 2x2 stride-1 max of P2 -> [15,15];  lvl1 plane (q,r) = P1[2q+i, 2r+j]
  P0 = 4x4 stride-1 max of P2 -> [13,13];  lvl0 plane    = P0
Output channel order: [lvl0: c][lvl1: c*4+q*2+r][lvl2: c*16+q*4+r].

Precision: the whole pipeline runs in bf16 (host converts f32->bf16 in,
bf16->f32 out).  Max-pooling is exact in any dtype, so the only error is
input rounding (<= 2^-9 relative), far inside the 2e-2 gate, and it
halves HBM traffic: 15.7 MB -> 7.8 MB per core (floor ~22 us).

Sharding: data-parallel over batch; each of 8 cores handles 2 samples as
4 tiles of 128 (b,c)-images on partitions.  Engine split per tile:
VectorE does the pairwise max trees plus the lvl1 gathers (all
tensor_tensor ops -- 1-port mode, never contends with GpSimd; a
same-operand tensor_max is a gather-copy at 2 elem/cyc), ScalarE stages
the 16 lvl2 planes (4 gather-copies), GpSimd issues the small SWDGE
stores.  Loads on the SP HWDGE ring (first tile as two halves, one per
HWDGE ring, so their transfers and sems overlap), one merged lvl2 store per
tile on the ACT ring (more stores -> HWDGE sem-lane stalls, measured
+3.5 us).  On the last tile the staging fans out (q0/q1 ACT, q2/q3 DVE)
and the three final store dispatches go on three different rings.

Measured: 51.5 us (f32 baseline) -> 37.8-38.3 us.  Breakdown: ~6.8 us
fixed NEFF preamble, ~8 us load-latency fill, ~21 us byte-bound DMA
stream (358 GB/s/core HBM), ~3.2 us final receipt + exit barrier.
"""

import sys

for _p in ("/opt/trn_rl_repo", "/opt/trn_rl_repo/concourse"):
    if _p not in sys.path:
        sys.path.insert(0, _p)

import numpy as np
import ml_dtypes

N_CORES = 8
BS, C, H, W = 16, 256, 64, 64
B_PER_CORE = BS // N_CORES  # 2
OH = OW = 13
CBLK = 2  # channel blocks of 128 per sample

_nc_cache = {}


def _build_nc(finalize=True):
    import concourse.bacc as bacc
    import concourse.mybir as mybir
    from concourse import tile
    from concourse.ap import AP as APc

    bf16 = mybir.dt.bfloat16
    # Bacc (not bare Bass): its finalize() runs generate_event_semaphores,
    # which splits multi-sem sync waits that walrus cannot encode.
    nc = bacc.Bacc("TRN2", target_bir_lowering=False)
    x = nc.dram_tensor("x", [B_PER_CORE, C, H, W], bf16, kind="ExternalInput")
    o = nc.dram_tensor("out", [B_PER_CORE, 21 * C, OH, OW], bf16, kind="ExternalOutput")

    def overlap(tap, start, dims):
        """Strided (possibly overlapping) free-dim view of a tile AP,
        starting at free-offset `start`.  Max 3 free dims (ISA limit)."""
        base = tap[:, start:]
        part = list(base.ap[0])
        return APc(
            tensor=base.tensor,
            offset=base.offset,
            ap=[part] + [[s, n] for (s, n) in dims],
        )

    with tile.TileContext(nc) as tc:
        with tc.tile_pool(name="sbuf", bufs=2) as pool:
            first = True
            for b in range(B_PER_CORE):
                for cb in range(CBLK):
                    cs = slice(cb * 128, (cb + 1) * 128)
                    r4 = pool.tile([128, 1024], bf16, tag="r4")
                    if first:
                        # Pipeline fill: two half loads, one per HWDGE ring
                        # (SP + ACT), so their transfers and completion sems
                        # overlap.  Two halves beat four quarters (fewer
                        # DMAs/ops/sems: 35.5 vs 37.8 us) and beat a single
                        # whole-tile load (36.3 us) -- the last chunk's sem
                        # latency dominates the fill either way, and fewer
                        # dispatches pull the later tile loads earlier.
                        first = False
                        for qt, (r0, rows) in enumerate(((0, 32), (32, 32))):
                            xq = pool.tile(
                                [128, rows * W], bf16, tag=f"xq{qt}", bufs=1
                            )
                            ring = nc.sync if qt % 2 == 0 else nc.scalar
                            ring.dma_start(
                                out=xq[:],
                                in_=x[b, cs, r0 : r0 + rows].rearrange(
                                    "c h w -> c (h w)"
                                ),
                            )
                            bq = pool.tile(
                                [128, rows * W // 2], bf16, tag=f"bq{qt}", bufs=1
                            )
                            xqv = xq.rearrange("p (a t c) -> p a t c", t=2, c=W)
                            nc.vector.tensor_max(
                                out=bq.rearrange("p (a c) -> p a c", c=W),
                                in0=xqv[:, :, 0, :],
                                in1=xqv[:, :, 1, :],
                            )
                            bqv = bq.rearrange("p (a t c) -> p a t c", t=2, c=W)
                            nc.vector.tensor_max(
                                out=r4[:, r0 * 16 : (r0 + rows) * 16].rearrange(
                                    "p (a c) -> p a c", c=W
                                ),
                                in0=bqv[:, :, 0, :],
                                in1=bqv[:, :, 1, :],
                            )
                    else:
                        # bufs=3 (no slot reuse among tiles 1-3): keeps loads
                        # early and waits trivial.  (Splitting any of these
                        # loads regresses 1-5 us: an extra DMA shifts the
                        # round-robin HWDGE sem-lane assignment for every
                        # later DMA and introduces event-sem stalls.)
                        xt = pool.tile([128, H * W], bf16, tag="xt", bufs=3)
                        nc.sync.dma_start(
                            out=xt[:],
                            in_=x[b, cs].rearrange("c h w -> c (h w)"),
                        )
                        b1 = pool.tile([128, 2048], bf16, tag="b1")
                        xv = xt.rearrange("p (a t c) -> p a t c", t=2, c=W)
                        nc.vector.tensor_max(
                            out=b1.rearrange("p (a c) -> p a c", c=W),
                            in0=xv[:, :, 0, :],
                            in1=xv[:, :, 1, :],
                        )
                        bv = b1.rearrange("p (a t c) -> p a t c", t=2, c=W)
                        nc.vector.tensor_max(
                            out=r4.rearrange("p (a c) -> p a c", c=W),
                            in0=bv[:, :, 0, :],
                            in1=bv[:, :, 1, :],
                        )
                    # 4-col max: [16,64] -> P2 [16,16].  First stage pairs
                    # (k, k+2) via run-of-2 step-1 views (packable 2x mode);
                    # d[g,u] = max(r4[4g+u], r4[4g+u+2]); p2[g] = max(d[g,0],
                    # d[g,1]).
                    c1 = pool.tile([128, 512], bf16, tag="c1")
                    nc.vector.tensor_max(
                        out=c1.rearrange("p (g u) -> p g u", u=2),
                        in0=overlap(r4, 0, [(4, 256), (1, 2)]),
                        in1=overlap(r4, 2, [(4, 256), (1, 2)]),
                    )
                    p2 = pool.tile([128, 256], bf16, tag="p2")
                    nc.vector.tensor_max(out=p2[:], in0=c1[:, 0::2], in1=c1[:, 1::2])

                    # bufs=3: with 2, tile t+2's compute waits on tile t's
                    # stores releasing the stage slot, which starves the
                    # store stream mid-kernel (measured 65% SDMA dip).
                    # bufs=4 measured neutral (38.1 us, within noise).
                    stage = pool.tile([128, 21 * OH * OW], bf16, tag="stage", bufs=3)

                    lvl2_dst = o[b, 1280 + cb * 2048 : 1280 + (cb + 1) * 2048].rearrange(
                        "(c f) h w -> c (f h w)", f=16
                    )
                    last = b == B_PER_CORE - 1 and cb == CBLK - 1

                    def lvl2_src(q):
                        return overlap(p2, q * 16, [(1, 4), (16, 13), (1, 13)])

                    def lvl2_dstap(q):
                        return stage[:, (5 + 4 * q) * 169 : (9 + 4 * q) * 169]

                    # lvl2: 16 shifted 13x13 windows of P2 -> stage[845:3549]
                    # (split over q: ISA mem patterns allow at most 3 free
                    # dims).  Mid tiles: all 4 q-chunks on ACT with ONE
                    # merged store -- the HWDGE dma_start dispatch overlaps
                    # the ACT pipe, and one dispatch keeps the sequencer
                    # stream short.  Last tile: q0/q1 on ACT (store as soon
                    # as both staged), q2/q3 as DVE same-operand tensor_max
                    # gathers AFTER the small-store producers so the SP-ring
                    # smalls dispatch early; their store goes on the idle SP
                    # ring, dispatched in parallel with ACT's.
                    if not last:
                        for q in range(4):
                            nc.scalar.copy(out=lvl2_dstap(q), in_=lvl2_src(q))
                        # (Routing these via nc.sync to defer them behind
                        # the loads -- implicit loads-first priority --
                        # measured 36.8 vs 36.0-36.4: no win.)
                        nc.scalar.dma_start(
                            out=lvl2_dst[:],
                            in_=stage[:, 5 * 169 : 21 * 169],
                        )
                    else:
                        for q in range(2):
                            nc.scalar.copy(out=lvl2_dstap(q), in_=lvl2_src(q))
                        nc.scalar.dma_start(
                            out=lvl2_dst[:, : 8 * 169],
                            in_=stage[:, 5 * 169 : 13 * 169],
                        )
                    # P1 = 2x2 stride-1 max of P2 -> [15,15]
                    t1 = pool.tile([128, 240], bf16, tag="t1")
                    p2m = p2.rearrange("p (h w) -> p h w", w=16)
                    nc.vector.tensor_max(
                        out=t1.rearrange("p (h w) -> p h w", w=15),
                        in0=p2m[:, :, 0:15],
                        in1=p2m[:, :, 1:16],
                    )
                    p1 = pool.tile([128, 225], bf16, tag="p1")
                    nc.vector.tensor_max(
                        out=p1[:], in0=t1[:, 0:225], in1=t1[:, 15:240]
                    )
                    # lvl1: 4 shifted 13x13 windows of P1 (stride 2) ->
                    # stage[169:845].  Same-operand tensor_max on DVE =
                    # gather-copy in packed 2x mode (~0.33 us/op).  (Staging
                    # any tile's lvl1 on ACT instead measured +2 us: the
                    # extra ACT-stream ops delay that tile's store path.)
                    for q in range(2):
                        src = lambda: overlap(p1, q * 30, [(2, 2), (15, 13), (1, 13)])
                        nc.vector.tensor_max(
                            out=stage[:, (1 + 2 * q) * 169 : (3 + 2 * q) * 169],
                            in0=src(),
                            in1=src(),
                        )
                    # P0 = 4x4 stride-1 max of P2 = 2x2 stride-2 max of P1
                    t2 = pool.tile([128, 195], bf16, tag="t2")
                    p1m = p1.rearrange("p (h w) -> p h w", w=15)
                    nc.vector.tensor_max(
                        out=t2.rearrange("p (h w) -> p h w", w=13),
                        in0=p1m[:, :, 0:13],
                        in1=p1m[:, :, 2:15],
                    )
                    nc.vector.tensor_max(
                        out=stage[:, 0:169], in0=t2[:, 0:169], in1=t2[:, 26:195]
                    )
                    # Small stores: SWDGE (GpSimd) keeps the HWDGE DMA count
                    # low.  On the last tile, spread the three remaining
                    # store dispatches across three rings (Pool/SP/ACT) so
                    # they fire concurrently instead of serializing ~0.7 us
                    # apiece behind one sequencer.
                    nc.gpsimd.dma_start(
                        out=o[b, 256 + cb * 512 : 256 + (cb + 1) * 512].rearrange(
                            "(c f) h w -> c (f h w)", f=4
                        ),
                        in_=stage[:, 169 : 5 * 169],
                    )
                    small = nc.sync if last else nc.gpsimd
                    small.dma_start(
                        out=o[b, cs].rearrange("c h w -> c (h w)"),
                        in_=stage[:, 0:169],
                    )
                    if last:
                        # Tail lvl2 q2/q3: DVE gathers (the scheduler runs
                        # them interleaved with the t1..p0 chain), store on
                        # the ACT ring which is idle after dma(q0q1).
                        # (Merging the last tile's two stores into one
                        # measured +3.5 us: the single store waits on all
                        # four chunks incl. the late DVE gathers.)
                        for q in (2, 3):
                            nc.vector.tensor_max(
                                out=lvl2_dstap(q), in0=lvl2_src(q), in1=lvl2_src(q)
                            )
                        nc.scalar.dma_start(
                            out=lvl2_dst[:, 8 * 169 :],
                            in_=stage[:, 13 * 169 : 21 * 169],
                        )
    if finalize:
        nc.finalize()
    return nc


def get_nc():
    if "nc" not in _nc_cache:
        _nc_cache["nc"] = _build_nc()
    return _nc_cache["nc"]


def kernel(x: np.ndarray, _trace: bool = False):
    from concourse.bass_utils import run_bass_kernel_spmd

    x = np.asarray(x)
    assert x.shape == (BS, C, H, W), x.shape
    xb = np.ascontiguousarray(x).astype(ml_dtypes.bfloat16)
    nc = get_nc()
    in_maps = [
        {"x": xb[c * B_PER_CORE : (c + 1) * B_PER_CORE]} for c in range(N_CORES)
    ]
    res = run_bass_kernel_spmd(
        nc, in_maps, core_ids=list(range(N_CORES)), trace=_trace
    )
    out = np.concatenate(
        [r["out"].astype(np.float32) for r in res.results], axis=0
    )
    if _trace:
        return out, res
    return out


# revision 43
# speedup vs baseline: 1.0094x; 1.0088x over previous
"""SPP (spatial pyramid pooling) kernel for Trainium2, 8 NeuronCores.

Input  x  : [16, 256, 64, 64] f32
Output    : [16, 5376, 13, 13] f32

Math: windows are 16x16 at stride 4 -> 13x13 window grid. Levels use
sub-cells of 16/8/4 pixels, all aligned to multiples of 4, so everything
reduces to the non-overlapping 4x4 block-max P2 [16,16] per (b,c) image:
  lvl2 plane (q,r) = P2[q+i, r+j]              (16 planes of 13x13)
  P1 = 2x2 stride-1 max of P2 -> [15,15];  lvl1 plane (q,r) = P1[2q+i, 2r+j]
  P0 = 4x4 stride-1 max of P2 -> [13,13];  lvl0 plane    = P0
Output channel order: [lvl0: c][lvl1: c*4+q*2+r][lvl2: c*16+q*4+r].

Precision: the whole pipeline runs in bf16 (host converts f32->bf16 in,
bf16->f32 out).  Max-pooling is exact in any dtype, so the only error is
input rounding (<= 2^-9 relative), far inside the 2e-2 gate, and it
halves HBM traffic: 15.7 MB -> 7.8 MB per core (floor ~22 us).

Sharding: data-parallel over batch; each of 8 cores handles 2 samples as
4 tiles of 128 (b,c)-images on partitions.  Engine split per tile:
VectorE does the pairwise max trees plus the lvl1 gathers (all
tensor_tensor ops -- 1-port mode, never contends with GpSimd; a
same-operand tensor_max is a gather-copy at 2 elem/cyc), ScalarE stages
the 16 lvl2 planes (4 gather-copies), GpSimd issues the small SWDGE
stores.  Loads on the SP HWDGE ring (first tile as two halves, one per
HWDGE ring, so their transfers and sems overlap), one merged lvl2 store per
tile on the ACT ring (more stores -> HWDGE sem-lane stalls, measured
+3.5 us).  On the last tile the staging fans out (q0/q1 ACT, q2/q3 DVE)
and the three final store dispatches go on three different rings.

Measured: 51.5 us (f32 baseline) -> 37.8-38.3 us.  Breakdown: ~6.8 us
fixed NEFF preamble, ~8 us load-latency fill, ~21 us byte-bound DMA
stream (358 GB/s/core HBM), ~3.2 us final receipt + exit barrier.
"""

import sys

for _p in ("/opt/trn_rl_repo", "/opt/trn_rl_repo/concourse"):
    if _p not in sys.path:
        sys.path.insert(0, _p)

import numpy as np
import ml_dtypes

N_CORES = 8
BS, C, H, W = 16, 256, 64, 64
B_PER_CORE = BS // N_CORES  # 2
OH = OW = 13
CBLK = 2  # channel blocks of 128 per sample

_nc_cache = {}


def _build_nc(finalize=True):
    import concourse.bacc as bacc
    import concourse.mybir as mybir
    from concourse import tile
    from concourse.ap import AP as APc

    bf16 = mybir.dt.bfloat16
    # Bacc (not bare Bass): its finalize() runs generate_event_semaphores,
    # which splits multi-sem sync waits that walrus cannot encode.
    nc = bacc.Bacc("TRN2", target_bir_lowering=False)
    x = nc.dram_tensor("x", [B_PER_CORE, C, H, W], bf16, kind="ExternalInput")
    o = nc.dram_tensor("out", [B_PER_CORE, 21 * C, OH, OW], bf16, kind="ExternalOutput")

    def overlap(tap, start, dims):
        """Strided (possibly overlapping) free-dim view of a tile AP,
        starting at free-offset `start`.  Max 3 free dims (ISA limit)."""
        base = tap[:, start:]
        part = list(base.ap[0])
        return APc(
            tensor=base.tensor,
            offset=base.offset,
            ap=[part] + [[s, n] for (s, n) in dims],
        )

    with tile.TileContext(nc) as tc:
        with tc.tile_pool(name="sbuf", bufs=2) as pool:
            first = True
            for b in range(B_PER_CORE):
                for cb in range(CBLK):
                    cs = slice(cb * 128, (cb + 1) * 128)
                    r4 = pool.tile([128, 1024], bf16, tag="r4")
                    if first:
                        # Pipeline fill: two half loads, one per HWDGE ring
                        # (SP + ACT), so their transfers and completion sems
                        # overlap.  Two halves beat four quarters (fewer
                        # DMAs/ops/sems: 35.5 vs 37.8 us) and beat a single
                        # whole-tile load (36.3 us) -- the last chunk's sem
                        # latency dominates the fill either way, and fewer
                        # dispatches pull the later tile loads earlier.
                        first = False
                        for qt, (r0, rows) in enumerate(((0, 32), (32, 32))):
                            xq = pool.tile(
                                [128, rows * W], bf16, tag=f"xq{qt}", bufs=1
                            )
                            ring = nc.sync if qt % 2 == 0 else nc.scalar
                            ring.dma_start(
                                out=xq[:],
                                in_=x[b, cs, r0 : r0 + rows].rearrange(
                                    "c h w -> c (h w)"
                                ),
                            )
                            bq = pool.tile(
                                [128, rows * W // 2], bf16, tag=f"bq{qt}", bufs=1
                            )
                            xqv = xq.rearrange("p (a t c) -> p a t c", t=2, c=W)
                            nc.vector.tensor_max(
                                out=bq.rearrange("p (a c) -> p a c", c=W),
                                in0=xqv[:, :, 0, :],
                                in1=xqv[:, :, 1, :],
                            )
                            bqv = bq.rearrange("p (a t c) -> p a t c", t=2, c=W)
                            nc.vector.tensor_max(
                                out=r4[:, r0 * 16 : (r0 + rows) * 16].rearrange(
                                    "p (a c) -> p a c", c=W
                                ),
                                in0=bqv[:, :, 0, :],
                                in1=bqv[:, :, 1, :],
                            )
                    else:
                        # bufs=3 (no slot reuse among tiles 1-3): keeps loads
                        # early and waits trivial.  (Splitting any of these
                        # loads regresses 1-5 us: an extra DMA shifts the
                        # round-robin HWDGE sem-lane assignment for every
                        # later DMA and introduces event-sem stalls.)
                        xt = pool.tile([128, H * W], bf16, tag="xt", bufs=3)
                        nc.sync.dma_start(
                            out=xt[:],
                            in_=x[b, cs].rearrange("c h w -> c (h w)"),
                        )
                        b1 = pool.tile([128, 2048], bf16, tag="b1")
                        xv = xt.rearrange("p (a t c) -> p a t c", t=2, c=W)
                        nc.vector.tensor_max(
                            out=b1.rearrange("p (a c) -> p a c", c=W),
                            in0=xv[:, :, 0, :],
                            in1=xv[:, :, 1, :],
                        )
                        bv = b1.rearrange("p (a t c) -> p a t c", t=2, c=W)
                        nc.vector.tensor_max(
                            out=r4.rearrange("p (a c) -> p a c", c=W),
                            in0=bv[:, :, 0, :],
                            in1=bv[:, :, 1, :],
                        )
                    # 4-col max: [16,64] -> P2 [16,16].  First stage pairs
                    # (k, k+2) via run-of-2 step-1 views (packable 2x mode);
                    # d[g,u] = max(r4[4g+u], r4[4g+u+2]); p2[g] = max(d[g,0],
                    # d[g,1]).
                    c1 = pool.tile([128, 512], bf16, tag="c1")
                    nc.vector.tensor_max(
                        out=c1.rearrange("p (g u) -> p g u", u=2),
                        in0=overlap(r4, 0, [(4, 256), (1, 2)]),
                        in1=overlap(r4, 2, [(4, 256), (1, 2)]),
                    )
                    p2 = pool.tile([128, 256], bf16, tag="p2")
                    nc.vector.tensor_max(out=p2[:], in0=c1[:, 0::2], in1=c1[:, 1::2])

                    # bufs=3: with 2, tile t+2's compute waits on tile t's
                    # stores releasing the stage slot, which starves the
                    # store stream mid-kernel (measured 65% SDMA dip).
                    # bufs=4 measured neutral (38.1 us, within noise).
                    stage = pool.tile([128, 21 * OH * OW], bf16, tag="stage", bufs=3)

                    lvl2_dst = o[b, 1280 + cb * 2048 : 1280 + (cb + 1) * 2048].rearrange(
                        "(c f) h w -> c (f h w)", f=16
                    )
                    last = b == B_PER_CORE - 1 and cb == CBLK - 1

                    def lvl2_src(q):
                        return overlap(p2, q * 16, [(1, 4), (16, 13), (1, 13)])

                    def lvl2_dstap(q):
                        return stage[:, (5 + 4 * q) * 169 : (9 + 4 * q) * 169]

                    # lvl2: 16 shifted 13x13 windows of P2 -> stage[845:3549]
                    # (split over q: ISA mem patterns allow at most 3 free
                    # dims).  Mid tiles: all 4 q-chunks on ACT with ONE
                    # merged store -- the HWDGE dma_start dispatch overlaps
                    # the ACT pipe, and one dispatch keeps the sequencer
                    # stream short.  Last tile: q0/q1 on ACT (store as soon
                    # as both staged), q2/q3 as DVE same-operand tensor_max
                    # gathers AFTER the small-store producers so the SP-ring
                    # smalls dispatch early; their store goes on the idle SP
                    # ring, dispatched in parallel with ACT's.
                    if not last:
                        for q in range(4):
                            nc.scalar.copy(out=lvl2_dstap(q), in_=lvl2_src(q))
                        # (Routing these via nc.sync to defer them behind
                        # the loads -- implicit loads-first priority --
                        # measured 36.8 vs 36.0-36.4: no win.)
                        nc.scalar.dma_start(
                            out=lvl2_dst[:],
                            in_=stage[:, 5 * 169 : 21 * 169],
                        )
                    else:
                        for q in range(2):
                            nc.scalar.copy(out=lvl2_dstap(q), in_=lvl2_src(q))
                        nc.scalar.dma_start(
                            out=lvl2_dst[:, : 8 * 169],
                            in_=stage[:, 5 * 169 : 13 * 169],
                        )
                    # P1 = 2x2 stride-1 max of P2 -> [15,15]
                    t1 = pool.tile([128, 240], bf16, tag="t1")
                    p2m = p2.rearrange("p (h w) -> p h w", w=16)
                    nc.vector.tensor_max(
                        out=t1.rearrange("p (h w) -> p h w", w=15),
                        in0=p2m[:, :, 0:15],
                        in1=p2m[:, :, 1:16],
                    )
                    p1 = pool.tile([128, 225], bf16, tag="p1")
                    nc.vector.tensor_max(
                        out=p1[:], in0=t1[:, 0:225], in1=t1[:, 15:240]
                    )
                    # lvl1: 4 shifted 13x13 windows of P1 (stride 2) ->
                    # stage[169:845].  Same-operand tensor_max on DVE =
                    # gather-copy in packed 2x mode (~0.33 us/op).  (Staging
                    # any tile's lvl1 on ACT instead measured +2 us: the
                    # extra ACT-stream ops delay that tile's store path.)
                    for q in range(2):
                        src = lambda: overlap(p1, q * 30, [(2, 2), (15, 13), (1, 13)])
                        nc.vector.tensor_max(
                            out=stage[:, (1 + 2 * q) * 169 : (3 + 2 * q) * 169],
                            in0=src(),
                            in1=src(),
                        )
                    # P0 = 4x4 stride-1 max of P2 = 2x2 stride-2 max of P1
                    t2 = pool.tile([128, 195], bf16, tag="t2")
                    p1m = p1.rearrange("p (h w) -> p h w", w=15)
                    nc.vector.tensor_max(
                        out=t2.rearrange("p (h w) -> p h w", w=13),
                        in0=p1m[:, :, 0:13],
                        in1=p1m[:, :, 2:15],
                    )
                    nc.vector.tensor_max(
                        out=stage[:, 0:169], in0=t2[:, 0:169], in1=t2[:, 26:195]
                    )
                    # Small stores: SWDGE (GpSimd) keeps the HWDGE DMA count
                    # low.  On the last tile, spread the three remaining
                    # store dispatches across three rings (Pool/SP/ACT) so
                    # they fire concurrently instead of serializing ~0.7 us
                    # apiece behind one sequencer.
                    # single_packet: batch the sub-512B-descriptor stores'
                    # descriptors into shared packets, amortizing the
                    # per-packet SDMA overhead on these small transfers.
                    nc.gpsimd.dma_start(
                        out=o[b, 256 + cb * 512 : 256 + (cb + 1) * 512].rearrange(
                            "(c f) h w -> c (f h w)", f=4
                        ),
                        in_=stage[:, 169 : 5 * 169],
                        single_packet=True,
                    )
                    small = nc.sync if last else nc.gpsimd
                    small.dma_start(
                        out=o[b, cs].rearrange("c h w -> c (h w)"),
                        in_=stage[:, 0:169],
                        single_packet=True,
                    )
                    if last:
                        # Tail lvl2 q2/q3: DVE gathers (the scheduler runs
                        # them interleaved with the t1..p0 chain), store on
                        # the ACT ring which is idle after dma(q0q1).
                        # (Merging the last tile's two stores into one
                        # measured +3.5 us: the single store waits on all
                        # four chunks incl. the late DVE gathers.)
                        for q in (2, 3):
                            nc.vector.tensor_max(
                                out=lvl2_dstap(q), in0=lvl2_src(q), in1=lvl2_src(q)
                            )
                        nc.scalar.dma_start(
                            out=lvl2_dst[:, 8 * 169 :],
                            in_=stage[:, 13 * 169 : 21 * 169],
                        )
    if finalize:
        nc.finalize()
    return nc


def get_nc():
    if "nc" not in _nc_cache:
        _nc_cache["nc"] = _build_nc()
    return _nc_cache["nc"]


def kernel(x: np.ndarray, _trace: bool = False):
    from concourse.bass_utils import run_bass_kernel_spmd

    x = np.asarray(x)
    assert x.shape == (BS, C, H, W), x.shape
    xb = np.ascontiguousarray(x).astype(ml_dtypes.bfloat16)
    nc = get_nc()
    in_maps = [
        {"x": xb[c * B_PER_CORE : (c + 1) * B_PER_CORE]} for c in range(N_CORES)
    ]
    res = run_bass_kernel_spmd(
        nc, in_maps, core_ids=list(range(N_CORES)), trace=_trace
    )
    out = np.concatenate(
        [r["out"].astype(np.float32) for r in res.results], axis=0
    )
    if _trace:
        return out, res
    return out
